# revision 30
# baseline (speedup 1.0000x reference)
"""IsoMaxPlus first-part kernel for TRN2 (8 NeuronCores, data-parallel on B).

out[b, c] = -|s| * sqrt(max(2 - 2 * <f_b/||f_b||, p_c/||p_c||>, 1e-12))

Host prep: per-core B-shard of features cast to fp8e4 and shipped twice —
natural layout packed [128, 64*512] (partition = row-within-128-block) for
the row-norm reduction, and transposed [512, 8192] for the matmul
stationary operand. Prototypes (replicated parameters) are normalized,
scaled by 16, cast fp8 and transposed on host into pnT8 [128, 4*1024].
Output is written packed [128, 64*1000] bf16; host unpacks, upcasts to
fp32 and applies the output sign in the same pass.

Device per core (BS=8192 rows, 64 blocks of 128):
  per 4-block batch: DVE affine_mul_reduce -> row norm^2 [128,4];
  ACT sqrt; DVE recip + (-2s^2/16) mul -> per-row psum scale.
  per block: 2x LDWEIGHTS + 4x DoubleRow fp8 matmul (256-deep
  contraction per MM, accumulate [128,1000] fp32 psum, 512/488 col
  split); ACT Sqrt(scale*x+bias) psum->sbuf bf16 fusing the feature
  normalization, prototype prescale and distance_scale.
  Loads prefetched one 32-block group ahead on the sync ring; 4-block
  output stores alternate between the gpsimd and sync rings (per-block
  for the last batch, so the tail isn't serialized on one store).
Engine busy/core (measured): ACT ~76us (64x [128,1000] Sqrt at the
1 elem/cycle/lane table rate is the floor) / DMA ~79 (25.4 MB) /
PE ~69 / DVE ~45 -> ACT/DMA ridge. HW exec ~105-120us depending on
the chip power state (P0 downclock under sustained load).
"""

import numpy as np
from contextlib import ExitStack

import ml_dtypes

import concourse.bass as bass
import concourse.tile as tile
from concourse import bacc, mybir
from concourse.bass import ts
from concourse.bass_utils import run_bass_kernel_spmd

N_CORES = 8
B, D, C = 65536, 512, 1000
CP = 1024                  # prototypes padded with zero rows
BS = B // N_CORES          # 8192 rows per core
NB = BS // 128             # 64 row blocks
KC = D // 128              # 4 contraction chunks
GRP = 32                   # blocks per feature-load group
FCH = 8                    # blocks per f_nat load chunk
OGRP = 4                   # blocks per output-store group
NBATCH = 8                 # blocks per norm batch
NSPLIT = (512, C - 512)    # psum halves (max free dim 512 per bank)
F32 = mybir.dt.float32
BF16 = mybir.dt.bfloat16
F8 = mybir.dt.float8e4
NPBF16 = np.dtype(ml_dtypes.bfloat16)
NPF8 = np.dtype(ml_dtypes.float8_e4m3)
PSCALE = 16.0              # prototype fp8 pre-scale (folded out of the sqrt)


def _emit(nc):
    fn_dram = nc.dram_tensor("f_nat", [128, NB * D], F8, kind="ExternalInput").ap()
    ft_dram = nc.dram_tensor("f_t", [D, BS], F8, kind="ExternalInput").ap()
    p_dram = nc.dram_tensor("pnt8", [128, KC * CP], F8, kind="ExternalInput").ap()
    s_dram = nc.dram_tensor("distance_scale", [1], F32, kind="ExternalInput").ap()
    o_dram = nc.dram_tensor("out", [128, NB * C], BF16, kind="ExternalOutput").ap()

    with tile.TileContext(nc) as tc, ExitStack() as ctx:
        singles = ctx.enter_context(tc.tile_pool(name="singles", bufs=1))
        nrm = ctx.enter_context(tc.tile_pool(name="nrm", bufs=6))
        sqp = ctx.enter_context(tc.tile_pool(name="sqp", bufs=2))
        ftp = ctx.enter_context(tc.tile_pool(name="ftp", bufs=2))
        fnp = ctx.enter_context(tc.tile_pool(name="fnp", bufs=2))
        opool = ctx.enter_context(tc.tile_pool(name="opool", bufs=3))
        mpsum = ctx.enter_context(tc.tile_pool(name="mpsum", bufs=4, space="PSUM"))

        # prototypes arrive pre-normalized/transposed/scaled from host
        pnT8 = singles.tile([128, KC, CP], F8)
        nc.scalar.dma_start(out=pnT8[:], in_=p_dram[:, :])

        # distance_scale -> per-partition constants 2*s^2 and -2*s^2/16
        s_b = singles.tile([128, 1], F32)
        nc.gpsimd.dma_start(out=s_b[:], in_=s_dram.to_broadcast([128, 1]))
        s2 = singles.tile([128, 1], F32)
        nc.vector.tensor_mul(s2[:], s_b[:], s_b[:])
        two_s2 = singles.tile([128, 1], F32)
        nc.vector.tensor_scalar_mul(two_s2[:], s2[:], 2.0)
        neg_two_s2 = singles.tile([128, 1], F32)
        nc.vector.tensor_scalar_mul(neg_two_s2[:], s2[:], -2.0 / PSCALE)

        # ---- main loop: 64 blocks of 128 feature rows ----
        NG = NB // GRP

        def issue_group_loads(g, pieces=1):
            # ftt first (PE needs it earliest); fnt in chunks so the first
            # norm batches can start before the whole group lands. For the
            # first group, ftt is loaded in column pieces interleaved with
            # fnt chunks so block 0 can start after ~1/4 of the bytes.
            ftt = ftp.tile([128, KC, GRP * 128], F8, tag="ftt")
            fnt = fnp.tile([128, GRP * D], F8, tag="fnt")
            pw = GRP * 128 // pieces
            fq = 0
            for p in range(pieces):
                for kc in range(KC):
                    nc.sync.dma_start(
                        out=ftt[:, kc, ts(p, pw)],
                        in_=ft_dram[ts(kc, 128),
                                    g * GRP * 128 + p * pw :
                                    g * GRP * 128 + (p + 1) * pw],
                    )
                while (fq + 1) * FCH * 128 <= (p + 1) * pw and fq < GRP // FCH:
                    nc.sync.dma_start(
                        out=fnt[:, ts(fq, FCH * D)],
                        in_=fn_dram[:, g * GRP * D + fq * FCH * D :
                                    g * GRP * D + (fq + 1) * FCH * D],
                    )
                    fq += 1
            return fnt, ftt

        ot = None
        scq = None
        pending = issue_group_loads(0, pieces=4)
        for g in range(NG):
            fnt, ftt = pending
            if g + 1 < NG:
                pending = issue_group_loads(g + 1)

            for j in range(GRP):
                ib = g * GRP + j
                jo = ib % OGRP
                jb = ib % NBATCH
                if jb == 0:
                    # norm batch for the next NBATCH blocks: one fused DVE
                    # square+reduce per block, then sqrt/recip/mul on [128,8]
                    n2q = nrm.tile([128, NBATCH], F32, tag="n2q")
                    for jj in range(NBATCH):
                        sqs = sqp.tile([128, D], BF16, tag="sqs")
                        nc.vector.affine_mul_reduce(
                            out=sqs[:],
                            accum_out=n2q[:, jj : jj + 1],
                            in0=fnt[:, ts(j + jj, D)],
                            in1=fnt[:, ts(j + jj, D)],
                            scale=1.0,
                            bias=0.0,
                        )
                    nc.scalar.sqrt(n2q[:], n2q[:])
                    rq = nrm.tile([128, NBATCH], F32, tag="rq")
                    nc.vector.reciprocal(rq[:], n2q[:])
                    scq = nrm.tile([128, NBATCH], F32, tag="scq")
                    nc.vector.tensor_scalar_mul(scq[:], rq[:], neg_two_s2[:])
                if jo == 0:
                    ot = opool.tile([128, OGRP * C], BF16, tag="ot")

                dots = mpsum.tile([128, C], F32)
                for h in range(KC // 2):
                    for lo, width in ((0, NSPLIT[0]), (NSPLIT[0], NSPLIT[1])):
                        nc.tensor.matmul(
                            dots[:, lo : lo + width],
                            ftt[:, 2 * h : 2 * h + 2, ts(j, 128)],
                            pnT8[:, 2 * h : 2 * h + 2, lo : lo + width],
                            start=(h == 0),
                            stop=(h == KC // 2 - 1),
                            perf_mode=mybir.MatmulPerfMode.DoubleRow,
                            skip_group_check=True,
                        )

                nc.scalar.activation(
                    ot[:, ts(jo, C)], dots[:], mybir.ActivationFunctionType.Sqrt,
                    bias=two_s2[:], scale=scq[:, jb : jb + 1],
                )
                if g == NG - 1 and j >= GRP - OGRP:
                    # final batch: store per block so the run's tail isn't
                    # serialized behind one large store
                    eng = nc.gpsimd if ib % 2 == 0 else nc.sync
                    eng.dma_start(
                        out=o_dram[:, ts(ib, C)], in_=ot[:, ts(jo, C)],
                    )
                elif jo == OGRP - 1:
                    go = ib // OGRP
                    eng = nc.gpsimd if go % 2 == 0 else nc.sync
                    eng.dma_start(
                        out=o_dram[:, ts(go, OGRP * C)], in_=ot[:],
                    )


def build():
    nc = bacc.Bacc("TRN2", target_bir_lowering=False, debug=False,
                   num_devices=N_CORES)
    _emit(nc)
    nc.compile()
    return nc


def _ensure_ntff_hook():
    """Dev-only: restore the axon NTFF profile hook that the trimmed agent
    image's antenv package lacks, so trace=True yields real HW timings."""
    import sys
    import types

    try:
        from antenv.axon_hooks import get_axon_ntff_profile_hook  # noqa: F401
        return
    except ImportError:
        pass
    from trn_agent_boot.trn_boot import _ntff_profile_via_ctypes

    hook = _ntff_profile_via_ctypes("/opt/axon/libaxon_pjrt.so")
    mod = types.ModuleType("antenv.axon_hooks")
    mod.get_axon_ntff_profile_hook = lambda: hook
    mod.set_axon_ntff_profile_hook = lambda h: None
    sys.modules["antenv.axon_hooks"] = mod


def _prep_pnt8(protos):
    """Normalize prototype rows, scale by PSCALE, cast fp8, pack [128, KC*CP]."""
    protos_pad = np.zeros((CP, D), dtype=np.float32)
    protos_pad[:C] = protos
    norm = np.maximum(np.linalg.norm(protos_pad, axis=1, keepdims=True), 1e-12)
    pn = (protos_pad / norm) * PSCALE
    # pnT8[p, kc, c] = pn[c, kc*128 + p]
    pnt = np.ascontiguousarray(pn.T.reshape(KC, 128, CP)).astype(NPF8)
    return np.ascontiguousarray(pnt.transpose(1, 0, 2)).reshape(128, KC * CP)


def _prep_core_inputs(feats, pnt8, dscale):
    """Shard + layout-pack one core's inputs (fp8 cast, dual layout)."""
    fb = feats.astype(NPF8)
    f_nat = np.ascontiguousarray(
        fb.reshape(NB, 128, D).transpose(1, 0, 2)
    ).reshape(128, NB * D)
    f_t = np.ascontiguousarray(fb.T)  # [D, BS]
    return {
        "f_nat": f_nat,
        "f_t": f_t,
        "pnt8": pnt8,
        "distance_scale": dscale,
    }


def run(inputs, trace=False):
    if trace:
        _ensure_ntff_hook()
    feats = np.ascontiguousarray(np.asarray(inputs["features"], dtype=np.float32))
    protos = np.ascontiguousarray(np.asarray(inputs["prototypes"], dtype=np.float32))
    dscale = np.ascontiguousarray(np.asarray(inputs["distance_scale"], dtype=np.float32))
    pnt8 = _prep_pnt8(protos)
    nc = build()
    in_maps = [
        _prep_core_inputs(feats[i * BS : (i + 1) * BS], pnt8, dscale)
        for i in range(N_CORES)
    ]
    res = run_bass_kernel_spmd(nc, in_maps, core_ids=list(range(N_CORES)),
                               trace=trace)

    def _unpack(o):
        # device emits positive |s|*sqrt(...) distances packed [128, NB*C]
        # in bf16; the sign flip rides the same upcast/unpack pass.
        o = np.asarray(o).reshape(128, NB, C).transpose(1, 0, 2)
        o = o.astype(np.float32)
        np.negative(o, out=o)
        return o.reshape(BS, C)

    out = np.concatenate([_unpack(r["out"]) for r in res.results], axis=0)
    return out, res


def kernel(**inputs) -> np.ndarray:
    out, _ = run(inputs, trace=False)
    return out
